# revision 65
# baseline (speedup 1.0000x reference)
"""GCN layer (gnn_message_passing) Trainium2 Bass kernel.

Problem: out[b,n,:] = relu( sum_r (mean_k padded[b, idx[b,r,n,k]]) @ W_r
                            + feat[b,n] @ W_self + bias )
  B=4, N=4096, D=O=128, R=4, K=16.

Strategy: shard (batch x N-half) across 8 cores -> no collectives.

Per-edge DMA gathers cost ~8ns/index of Q7 descriptor generation on the
Pool engine (~1.1ms/core floor), so the neighbor aggregation is instead
a dense matmul with a host-packed multi-hot count matrix:

  out.T[o, n] = sum_i T_all[i, o] * M[i, n] + W_self.T @ X.T + bias

T_all = [padded @ W_r / K, r=0..3] stacked ([4*4224, 128] bf16, built
on-device on the PE). M[r*4224 + row, n] = #{k : idx[b,r,n,k] == row}
in fp8e4 (counts 0..16 are exact; halves HBM traffic vs bf16). M
streams at line rate into the tensor engine; Pool is unused.

Loop order is i-tile-major with all 4 node-chunk accumulators live in
separate PSUM banks, so each T_all tile is loaded once per 4 matmuls.
ReLU+bias fuse on ACT (bias per-partition, exact f32); output leaves
transposed [o, n] and the host re-transposes.
"""

import numpy as np
import ml_dtypes

import concourse.bacc as bacc
import concourse.mybir as mybir
from concourse.tile import TileContext
from concourse.bass_utils import run_bass_kernel_spmd

B, N, D = 4, 4096, 128
R, K, O = 4, 16, 128
NCORES = 8
NH = N // 2            # nodes per core
CH = 512               # nodes per chunk (one PSUM bank)
NCH = NH // CH         # chunks per core (4)
RT = 4224              # padded table rows per relation (33 * 128)
TILES = R * (RT // 128)   # 132 i-tiles
GS = [6, 6] + [12] * 9 + [6, 4, 2]   # i-tiles per M load group: small first
                                     # groups (early matmul start), descending
                                     # tail taper (short post-stream tail)

M_DT = mybir.dt.float8e4
M_NP = ml_dtypes.float8_e4m3

_cache = {}


def _build():
    nc = bacc.Bacc("TRN2")
    tblT = nc.dram_tensor("tblT", [128, RT], mybir.dt.bfloat16, kind="ExternalInput")
    selfT = nc.dram_tensor("selfT", [128, NH], mybir.dt.bfloat16, kind="ExternalInput")
    w = nc.dram_tensor("w", [128, R + 1, O], mybir.dt.bfloat16, kind="ExternalInput")
    bias = nc.dram_tensor("bias", [128, 1], mybir.dt.float32, kind="ExternalInput")
    m_in = nc.dram_tensor("m", [128, TILES, NH], M_DT, kind="ExternalInput")
    out = nc.dram_tensor("out", [128, NCH, CH], mybir.dt.float32,
                         kind="ExternalOutput")

    with TileContext(nc) as tc:
        with (
            tc.tile_pool(name="const", bufs=1) as cpool,
            tc.tile_pool(name="m", bufs=3) as mpool,
            tc.tile_pool(name="o", bufs=4) as opool,
            tc.tile_pool(name="pst", bufs=2, space="PSUM") as ptpool,
            tc.tile_pool(name="ps", bufs=1, space="PSUM") as pspool,
        ):
            # queue order: tiny w first, then the first (small) M group so
            # the M stream — the critical path — starts immediately; the
            # table rides between M groups. The per-core table is rotated
            # host-side so every core's own nodes sit at rows 1..NH+1,
            # making the self term a fixed tblT slice (no separate tensor).
            w_sb = cpool.tile([128, R + 1, O], mybir.dt.bfloat16)
            nc.sync.dma_start(w_sb[:], w[:])
            tblT_sb = cpool.tile([128, RT], mybir.dt.bfloat16)
            nc.sync.dma_start(tblT_sb[:], tblT[:])
            selfT_sb = cpool.tile([128, NH], mybir.dt.bfloat16)
            nc.sync.dma_start(selfT_sb[:], selfT[:])
            bias_sb = cpool.tile([128, 1], mybir.dt.float32)
            nc.sync.dma_start(bias_sb[:], bias[:])

            ps = [pspool.tile([128, CH], mybir.dt.float32, name=f"ps{i}",
                              tag=f"ps{i}")
                  for i in range(NCH)]
            # T_all[i, t, o] in fp8e4, built at 16x scale (relation kernels
            # carry no /K; ACT scale=1/16 undoes it). One wide matmul per
            # column tile builds all 4 relations; runs while M streams in.
            t_all = cpool.tile([128, TILES, O], M_DT)
            nt = RT // 128
            for tt in range(nt):
                pt = ptpool.tile([128, R, O], mybir.dt.float32)
                nc.tensor.matmul(
                    pt[:], tblT_sb[:, tt * 128:(tt + 1) * 128],
                    w_sb[:, 0:R, :],
                    start=True, stop=True,
                )
                nc.vector.tensor_copy(t_all[:, tt:TILES:nt, :], pt[:])

            t0 = 0
            for g, gsz in enumerate(GS):
                m_sb = mpool.tile([128, gsz, NH], M_DT, name=f"m{gsz}",
                                  tag=f"m{gsz}")
                nc.sync.dma_start(m_sb[:], m_in[:, t0:t0 + gsz, :])
                if g == 0:
                    for ch in range(NCH):
                        nc.tensor.matmul(
                            ps[ch][:], w_sb[:, R, :],
                            selfT_sb[:, ch * CH:(ch + 1) * CH],
                            start=True, stop=False, skip_group_check=True,
                        )
                for tl in range(0, gsz, 2):
                    t = t0 + tl
                    for ch in range(NCH):
                        # fp8 DoubleRow: 2 i-tiles (256 contraction rows)
                        # per matmul; pair interleave is order-invariant
                        # since both operands index pairs identically.
                        nc.tensor.matmul(
                            ps[ch][:], t_all[:, t:t + 2, :],
                            m_sb[:, tl:tl + 2, ch * CH:(ch + 1) * CH],
                            start=False, stop=(t == TILES - 2),
                            skip_group_check=True,
                            perf_mode=mybir.MatmulPerfMode.DoubleRow,
                        )
                t0 += gsz
            for ch in range(NCH):
                out_sb = opool.tile([128, CH], mybir.dt.float32)
                nc.scalar.activation(
                    out_sb[:], ps[ch][:], mybir.ActivationFunctionType.Relu,
                    bias=bias_sb[:], scale=1.0 / 16.0,
                )
                nc.sync.dma_start(out[:, ch, :], out_sb[:])

    nc.compile()
    return nc


def _prep_inputs(node_features, neighbor_indices, relation_kernels, self_kernel, bias):
    """Host-side shard/layout prep. Returns per-core input maps."""
    nf = np.asarray(node_features)
    idx = np.asarray(neighbor_indices)

    # relation kernels carry 16x scale (the /K mean and a 16x fp8-range
    # boost cancel); self kernel gets the 16x boost explicitly. The ACT
    # applies 1/16 to the PSUM total.
    w = np.zeros((128, R + 1, O), dtype=ml_dtypes.bfloat16)
    for r in range(R):
        w[:, r, :] = np.asarray(relation_kernels)[r].astype(ml_dtypes.bfloat16)
    w[:, R, :] = (np.asarray(self_kernel) * 16.0).astype(ml_dtypes.bfloat16)
    bias_col = np.asarray(bias).astype(np.float32).reshape(128, 1)

    nfT = [nf[b].T.astype(ml_dtypes.bfloat16) for b in range(B)]

    in_maps = []
    cols = np.repeat(np.arange(NH, dtype=np.int64), K)
    for c in range(NCORES):
        b, h = divmod(c, 2)
        base = h * NH
        # rotate table rows so this core's own nodes sit at rows 1..NH+1
        # (the self term then reads a fixed tblT slice); M rows rotate
        # identically, so the product is exactly unchanged.
        t = np.zeros((128, RT), dtype=ml_dtypes.bfloat16)
        t[:, 1:N + 1] = np.roll(nfT[b], -base, axis=1)
        cnt = np.zeros((R * RT, NH), dtype=np.uint8)
        for r in range(R):
            iv = idx[b, r, base:base + NH, :].astype(np.int64)
            loc = np.where(iv > 0, (iv - 1 - base) % N + 1, 0)
            np.add.at(cnt, ((r * RT + loc).ravel(), cols), 1)
        m = cnt.reshape(TILES, 128, NH).transpose(1, 0, 2)
        in_maps.append({
            "tblT": t,
            "selfT": np.ascontiguousarray(t[:, 1:1 + NH]),
            "w": w,
            "bias": bias_col,
            "m": np.ascontiguousarray(m).astype(M_NP),
        })
    return in_maps


def _run(in_maps, **kw):
    if "nc" not in _cache:
        _cache["nc"] = _build()
    return run_bass_kernel_spmd(_cache["nc"], in_maps, core_ids=list(range(NCORES)), **kw)


def _assemble(results):
    out = np.empty((B, N, O), dtype=np.float32)
    for c in range(NCORES):
        b, h = divmod(c, 2)
        o = results[c]["out"]  # [128, NCH, CH] = [o, ch, n]
        out[b, h * NH:(h + 1) * NH, :] = o.transpose(1, 2, 0).reshape(NH, O)
    return out


def kernel(node_features, neighbor_indices, relation_kernels, self_kernel, bias):
    in_maps = _prep_inputs(node_features, neighbor_indices, relation_kernels,
                           self_kernel, bias)
    res = _run(in_maps)
    return _assemble(res.results)


# revision 66
# speedup vs baseline: 1.0704x; 1.0704x over previous
"""GCN layer (gnn_message_passing) Trainium2 Bass kernel.

Problem: out[b,n,:] = relu( sum_r (mean_k padded[b, idx[b,r,n,k]]) @ W_r
                            + feat[b,n] @ W_self + bias )
  B=4, N=4096, D=O=128, R=4, K=16.

Strategy: shard (batch x N-half) across 8 cores -> no collectives.

Per-edge DMA gathers cost ~8ns/index of Q7 descriptor generation on the
Pool engine (~1.1ms/core floor), so the neighbor aggregation is instead
a dense matmul with a host-packed multi-hot count matrix:

  out.T[o, n] = sum_i T_all[i, o] * M[i, n] + W_self.T @ X.T + bias

T_all = [padded @ W_r / K, r=0..3] stacked ([4*4224, 128] bf16, built
on-device on the PE). M[r*4224 + row, n] = #{k : idx[b,r,n,k] == row}
in fp8e4 (counts 0..16 are exact; halves HBM traffic vs bf16). M
streams at line rate into the tensor engine; Pool is unused.

Loop order is i-tile-major with all 4 node-chunk accumulators live in
separate PSUM banks, so each T_all tile is loaded once per 4 matmuls.
ReLU+bias fuse on ACT (bias per-partition, exact f32); output leaves
transposed [o, n] and the host re-transposes.
"""

import numpy as np
import ml_dtypes

import concourse.bacc as bacc
import concourse.mybir as mybir
from concourse.tile import TileContext
from concourse.bass_utils import run_bass_kernel_spmd

B, N, D = 4, 4096, 128
R, K, O = 4, 16, 128
NCORES = 8
NH = N // 2            # nodes per core
CH = 512               # nodes per chunk (one PSUM bank)
NCH = NH // CH         # chunks per core (4)
RT = 4224              # padded table rows per relation (33 * 128)
TILES = R * (RT // 128)   # 132 i-tiles
GS = [6, 6] + [12] * 9 + [4, 4, 4]   # i-tiles per M load group: small first
                                     # groups (early matmul start) and small
                                     # last groups (short post-stream tail)

M_DT = mybir.dt.float8e4
M_NP = ml_dtypes.float8_e4m3

_cache = {}


def _build():
    nc = bacc.Bacc("TRN2")
    tblT = nc.dram_tensor("tblT", [128, RT], mybir.dt.bfloat16, kind="ExternalInput")
    selfT = nc.dram_tensor("selfT", [128, NH], mybir.dt.bfloat16, kind="ExternalInput")
    w = nc.dram_tensor("w", [128, R + 1, O], mybir.dt.bfloat16, kind="ExternalInput")
    bias = nc.dram_tensor("bias", [128, 1], mybir.dt.float32, kind="ExternalInput")
    m_in = nc.dram_tensor("m", [128, TILES, NH], M_DT, kind="ExternalInput")
    out = nc.dram_tensor("out", [128, NCH, CH], mybir.dt.float32,
                         kind="ExternalOutput")

    with TileContext(nc) as tc:
        with (
            tc.tile_pool(name="const", bufs=1) as cpool,
            tc.tile_pool(name="m", bufs=3) as mpool,
            tc.tile_pool(name="o", bufs=4) as opool,
            tc.tile_pool(name="pst", bufs=2, space="PSUM") as ptpool,
            tc.tile_pool(name="ps", bufs=1, space="PSUM") as pspool,
        ):
            # queue order: tiny w first, then the first (small) M group so
            # the M stream — the critical path — starts immediately; the
            # table rides between M groups. The per-core table is rotated
            # host-side so every core's own nodes sit at rows 1..NH+1,
            # making the self term a fixed tblT slice (no separate tensor).
            w_sb = cpool.tile([128, R + 1, O], mybir.dt.bfloat16)
            nc.sync.dma_start(w_sb[:], w[:])
            tblT_sb = cpool.tile([128, RT], mybir.dt.bfloat16)
            nc.sync.dma_start(tblT_sb[:], tblT[:])
            selfT_sb = cpool.tile([128, NH], mybir.dt.bfloat16)
            nc.sync.dma_start(selfT_sb[:], selfT[:])
            bias_sb = cpool.tile([128, 1], mybir.dt.float32)
            nc.sync.dma_start(bias_sb[:], bias[:])

            ps = [pspool.tile([128, CH], mybir.dt.float32, name=f"ps{i}",
                              tag=f"ps{i}")
                  for i in range(NCH)]
            # T_all[i, t, o] in fp8e4, built at 16x scale (relation kernels
            # carry no /K; ACT scale=1/16 undoes it). One wide matmul per
            # column tile builds all 4 relations; runs while M streams in.
            t_all = cpool.tile([128, TILES, O], M_DT)
            nt = RT // 128
            for tt in range(nt):
                pt = ptpool.tile([128, R, O], mybir.dt.float32)
                nc.tensor.matmul(
                    pt[:], tblT_sb[:, tt * 128:(tt + 1) * 128],
                    w_sb[:, 0:R, :],
                    start=True, stop=True,
                )
                nc.vector.tensor_copy(t_all[:, tt:TILES:nt, :], pt[:])

            t0 = 0
            for g, gsz in enumerate(GS):
                m_sb = mpool.tile([128, gsz, NH], M_DT, name=f"m{gsz}",
                                  tag=f"m{gsz}")
                nc.sync.dma_start(m_sb[:], m_in[:, t0:t0 + gsz, :])
                if g == 0:
                    for ch in range(NCH):
                        nc.tensor.matmul(
                            ps[ch][:], w_sb[:, R, :],
                            selfT_sb[:, ch * CH:(ch + 1) * CH],
                            start=True, stop=False, skip_group_check=True,
                        )
                for tl in range(0, gsz, 2):
                    t = t0 + tl
                    for ch in range(NCH):
                        # fp8 DoubleRow: 2 i-tiles (256 contraction rows)
                        # per matmul; pair interleave is order-invariant
                        # since both operands index pairs identically.
                        nc.tensor.matmul(
                            ps[ch][:], t_all[:, t:t + 2, :],
                            m_sb[:, tl:tl + 2, ch * CH:(ch + 1) * CH],
                            start=False, stop=(t == TILES - 2),
                            skip_group_check=True,
                            perf_mode=mybir.MatmulPerfMode.DoubleRow,
                        )
                t0 += gsz
            for ch in range(NCH):
                out_sb = opool.tile([128, CH], mybir.dt.float32)
                nc.scalar.activation(
                    out_sb[:], ps[ch][:], mybir.ActivationFunctionType.Relu,
                    bias=bias_sb[:], scale=1.0 / 16.0,
                )
                nc.sync.dma_start(out[:, ch, :], out_sb[:])

    nc.compile()
    return nc


def _prep_inputs(node_features, neighbor_indices, relation_kernels, self_kernel, bias):
    """Host-side shard/layout prep. Returns per-core input maps."""
    nf = np.asarray(node_features)
    idx = np.asarray(neighbor_indices)

    # relation kernels carry 16x scale (the /K mean and a 16x fp8-range
    # boost cancel); self kernel gets the 16x boost explicitly. The ACT
    # applies 1/16 to the PSUM total.
    w = np.zeros((128, R + 1, O), dtype=ml_dtypes.bfloat16)
    for r in range(R):
        w[:, r, :] = np.asarray(relation_kernels)[r].astype(ml_dtypes.bfloat16)
    w[:, R, :] = (np.asarray(self_kernel) * 16.0).astype(ml_dtypes.bfloat16)
    bias_col = np.asarray(bias).astype(np.float32).reshape(128, 1)

    nfT = [nf[b].T.astype(ml_dtypes.bfloat16) for b in range(B)]

    in_maps = []
    cols = np.repeat(np.arange(NH, dtype=np.int64), K)
    for c in range(NCORES):
        b, h = divmod(c, 2)
        base = h * NH
        # rotate table rows so this core's own nodes sit at rows 1..NH+1
        # (the self term then reads a fixed tblT slice); M rows rotate
        # identically, so the product is exactly unchanged.
        t = np.zeros((128, RT), dtype=ml_dtypes.bfloat16)
        t[:, 1:N + 1] = np.roll(nfT[b], -base, axis=1)
        cnt = np.zeros((R * RT, NH), dtype=np.uint8)
        for r in range(R):
            iv = idx[b, r, base:base + NH, :].astype(np.int64)
            loc = np.where(iv > 0, (iv - 1 - base) % N + 1, 0)
            np.add.at(cnt, ((r * RT + loc).ravel(), cols), 1)
        m = cnt.reshape(TILES, 128, NH).transpose(1, 0, 2)
        in_maps.append({
            "tblT": t,
            "selfT": np.ascontiguousarray(t[:, 1:1 + NH]),
            "w": w,
            "bias": bias_col,
            "m": np.ascontiguousarray(m).astype(M_NP),
        })
    return in_maps


def _run(in_maps, **kw):
    if "nc" not in _cache:
        _cache["nc"] = _build()
    return run_bass_kernel_spmd(_cache["nc"], in_maps, core_ids=list(range(NCORES)), **kw)


def _assemble(results):
    out = np.empty((B, N, O), dtype=np.float32)
    for c in range(NCORES):
        b, h = divmod(c, 2)
        o = results[c]["out"]  # [128, NCH, CH] = [o, ch, n]
        out[b, h * NH:(h + 1) * NH, :] = o.transpose(1, 2, 0).reshape(NH, O)
    return out


def kernel(node_features, neighbor_indices, relation_kernels, self_kernel, bias):
    in_maps = _prep_inputs(node_features, neighbor_indices, relation_kernels,
                           self_kernel, bias)
    res = _run(in_maps)
    return _assemble(res.results)
